# revision 4
# baseline (speedup 1.0000x reference)
"""Causal single-head attention on 8 TRN2 NeuronCores.

Problem: x[4, 2048, 1024], Wq/Wk/Wv[1024, 1024] fp32.
  q,k,v = x@W*; scores = q@k^T; masked = scores*tril + (1-tril)*(-1e9)
  attn = softmax(masked/sqrt(1024)); out = attn@v.

Sharding: 2 cores per batch. Query rows are split into eight 256-row
blocks; parity-0 cores take blocks {0,2,4,6}, parity-1 {1,3,5,7}, so
each core's 4 slots attend to exactly (1,2,3,4) 512-wide key panels —
identical program on all 8 cores (SPMD), balanced causal work, no
collectives. Each core computes k/v projections for its whole batch
(k^T and v bounce through DRAM scratch), q projection for its 1024
rows, then block-wise masked softmax(QK^T/32)V. Matmuls run in
float32r (~13-bit mantissa, 4x faster than fp32 on the PE).

Host side: slices x per core, pre-transposes x and xq (so the kernel
needs no PE transposes for projections), builds additive causal mask
biases for each slot's last key panel, and scatters the per-core
outputs back into the full [4, 2048, 1024] tensor.
"""
import sys

if "/opt/trn_rl_repo" not in sys.path:
    sys.path.insert(0, "/opt/trn_rl_repo")

import numpy as np

import concourse.bass as bass
import concourse.tile as tile
from concourse import bacc, mybir
from concourse.bass_utils import run_bass_kernel_spmd
from concourse.masks import make_identity

dt = mybir.dt

B, S, D = 4, 2048, 1024
P = 128
NEG = -1.0e9
QBLK = 256            # query rows per slot
KPAN = 512            # key panel width
NSLOT = 4             # slots per core
COUNTS = (1, 2, 3, 4)  # key panels per slot (both parities)
SCALE = 1.0 / 32.0    # 1/sqrt(D)

_nc_cache = {}


def build_nc(reps=1):
    """Build the per-core Bass program (same NEFF for all 8 cores)."""
    nc = bacc.Bacc(None, target_bir_lowering=False, debug=False)

    xt = nc.dram_tensor("xt", [D, S], dt.float32, kind="ExternalInput")
    xqt = nc.dram_tensor("xqt", [D, NSLOT * QBLK], dt.float32, kind="ExternalInput")
    wq = nc.dram_tensor("wq", [D, D], dt.float32, kind="ExternalInput")
    wk = nc.dram_tensor("wk", [D, D], dt.float32, kind="ExternalInput")
    wv = nc.dram_tensor("wv", [D, D], dt.float32, kind="ExternalInput")
    # additive causal bias for each slot's LAST key panel, laid out
    # [p, slot, qsub, key] with q-local row = qsub*128 + p
    mb = nc.dram_tensor("mb", [P, NSLOT, 2, KPAN], dt.float32, kind="ExternalInput")
    out = nc.dram_tensor("out", [NSLOT * QBLK, D], dt.float32, kind="ExternalOutput")

    # DRAM scratch for k^T and v (already f32r-rounded bits)
    kt_d = nc.dram_tensor("kt_d", [P, 8, S], dt.float32r)
    v_d = nc.dram_tensor("v_d", [P, S // P, D], dt.float32r)

    DC = D // P  # 8 contraction chunks

    def proj_matmuls(psum_t, lhs_r, rhs_r):
        for dc in range(DC):
            nc.tensor.matmul(
                psum_t, lhs_r[:, dc], rhs_r[:, dc],
                start=(dc == 0), stop=(dc == DC - 1),
            )

    def copy_eng(i, out_ap, in_ap):
        if i % 2 == 0:
            nc.vector.tensor_copy(out_ap, in_ap)
        else:
            nc.scalar.copy(out_ap, in_ap)

    with tile.TileContext(nc) as tc:
        with (
            tc.tile_pool(name="const", bufs=1) as const,
            tc.tile_pool(name="persist", bufs=1) as persist,
        ):
            ident = const.tile([P, P], dt.float32)
            make_identity(nc, ident)

            qt_r = persist.tile([P, DC, NSLOT * QBLK], dt.float32r)

            def body():
                # ---- Phase Q: q^T projection for my 1024 query rows ----
                with (
                    tc.tile_pool(name="qstage", bufs=2) as qstage,
                    tc.tile_pool(name="qround", bufs=1) as qround,
                    tc.tile_pool(name="psum_p", bufs=4, space="PSUM") as psum_p,
                ):
                    xqs = qstage.tile([P, DC, 1024], dt.float32, tag="st")
                    nc.sync.dma_start(
                        xqs[:], xqt.rearrange("(dc p) t -> p dc t", p=P))
                    wqs = qstage.tile([P, DC, 1024], dt.float32, tag="st")
                    nc.sync.dma_start(
                        wqs[:], wq.rearrange("(dc p) m -> p dc m", p=P))
                    xq_r = qround.tile([P, DC, 1024], dt.float32r, tag="xq")
                    wq_r = qround.tile([P, DC, 1024], dt.float32r, tag="wq")
                    nc.vector.tensor_copy(xq_r[:], xqs[:])
                    nc.vector.tensor_copy(wq_r[:], wqs[:])
                    for do in range(DC):
                        for th in range(2):
                            ps = psum_p.tile([P, 512], dt.float32, tag="pp")
                            proj_matmuls(
                                ps,
                                wq_r[:, :, do * P:(do + 1) * P],
                                xq_r[:, :, th * 512:(th + 1) * 512],
                            )
                            copy_eng(do + th,
                                qt_r[:, do, th * 512:(th + 1) * 512], ps[:])

                # ---- Phase KV: k^T and v for all 2048 keys -> DRAM ----
                with (
                    tc.tile_pool(name="xtpool", bufs=1) as xtpool,
                    tc.tile_pool(name="psum_kv", bufs=4, space="PSUM") as psum_kv,
                ):
                    xt_r = xtpool.tile([P, DC, S], dt.float32r)
                    with tc.tile_pool(name="kvstage", bufs=2) as kvs:
                        for h in range(2):
                            xts = kvs.tile([P, DC, 1024], dt.float32, tag="st")
                            nc.sync.dma_start(
                                xts[:],
                                xt.rearrange("(dc p) t -> p dc t", p=P)
                                [:, :, h * 1024:(h + 1) * 1024])
                            nc.vector.tensor_copy(
                                xt_r[:, :, h * 1024:(h + 1) * 1024], xts[:])

                    for w_dram, which in ((wk, "k"), (wv, "v")):
                        with (
                            tc.tile_pool(name="wpool", bufs=1) as wpool,
                            tc.tile_pool(name="wstage", bufs=2) as wstage,
                            tc.tile_pool(name="ostage", bufs=4) as ostage,
                        ):
                            w_r = wpool.tile([P, DC, 1024], dt.float32r)
                            for h in range(2):
                                ws = wstage.tile([P, DC, 512], dt.float32, tag="ws")
                                nc.sync.dma_start(
                                    ws[:],
                                    w_dram.rearrange("(dc p) m -> p dc m", p=P)
                                    [:, :, h * 512:(h + 1) * 512])
                                nc.vector.tensor_copy(
                                    w_r[:, :, h * 512:(h + 1) * 512], ws[:])
                            if which == "k":
                                # kt[dout, key]: lhsT=wk chunk, rhs=xt chunk
                                for do in range(DC):
                                    for kq in range(S // 512):
                                        ps = psum_kv.tile(
                                            [P, 512], dt.float32, tag="pp")
                                        proj_matmuls(
                                            ps,
                                            w_r[:, :, do * P:(do + 1) * P],
                                            xt_r[:, :, kq * 512:(kq + 1) * 512],
                                        )
                                        st = ostage.tile(
                                            [P, 512], dt.float32r, tag="os")
                                        copy_eng(do + kq, st[:], ps[:])
                                        nc.sync.dma_start(
                                            kt_d[:, do, kq * 512:(kq + 1) * 512],
                                            st[:])
                            else:
                                # v[key, dout]: lhsT=xt key chunk, rhs=wv
                                for kc in range(S // P):
                                    for dh in range(2):
                                        ps = psum_kv.tile(
                                            [P, 512], dt.float32, tag="pp")
                                        proj_matmuls(
                                            ps,
                                            xt_r[:, :, kc * P:(kc + 1) * P],
                                            w_r[:, :, dh * 512:(dh + 1) * 512],
                                        )
                                        st = ostage.tile(
                                            [P, 512], dt.float32r, tag="os")
                                        copy_eng(kc + dh, st[:], ps[:])
                                        nc.sync.dma_start(
                                            v_d[:, kc, dh * 512:(dh + 1) * 512],
                                            st[:])

                # ---- Phase A: blockwise masked softmax(QK^T/32) V ----
                with (
                    tc.tile_pool(name="attn", bufs=1) as attn,
                    tc.tile_pool(name="ktpool", bufs=2) as ktpool,
                    tc.tile_pool(name="ptpool", bufs=1) as ptpool,
                    tc.tile_pool(name="vpool", bufs=2) as vpool,
                    tc.tile_pool(name="opool", bufs=4) as opool,
                    tc.tile_pool(name="small", bufs=24) as small,
                    tc.tile_pool(name="psum_s", bufs=2, space="PSUM") as psum_s,
                    tc.tile_pool(name="psum_t", bufs=2, space="PSUM") as psum_t,
                    tc.tile_pool(name="psum_c", bufs=4, space="PSUM") as psum_c,
                ):
                    masks = attn.tile([P, NSLOT, 2, KPAN], dt.float32)
                    nc.sync.dma_start(masks[:], mb[:])
                    scores = [
                        attn.tile([P, 2, (s + 1) * KPAN], dt.float32,
                                  tag=f"sc{s}", name=f"scores{s}")
                        for s in range(NSLOT)
                    ]
                    # panel-major scores: k^T panel read once
                    for p in range(NSLOT):
                        ktp = ktpool.tile([P, DC, KPAN], dt.float32r, tag="kt")
                        nc.sync.dma_start(
                            ktp[:], kt_d[:, :, p * KPAN:(p + 1) * KPAN])
                        for s in range(p, NSLOT):
                            for qs in range(2):
                                ps = psum_s.tile([P, KPAN], dt.float32, tag="ps")
                                for dc in range(DC):
                                    nc.tensor.matmul(
                                        ps,
                                        qt_r[:, dc,
                                             s * QBLK + qs * P:
                                             s * QBLK + (qs + 1) * P],
                                        ktp[:, dc],
                                        start=(dc == 0), stop=(dc == DC - 1),
                                    )
                                dst = scores[s][:, qs, p * KPAN:(p + 1) * KPAN]
                                if p == s:  # this slot's last panel: add mask
                                    nc.vector.tensor_tensor(
                                        dst, ps[:], masks[:, s, qs, :],
                                        op=mybir.AluOpType.add)
                                else:
                                    copy_eng(s + qs, dst, ps[:])

                    for s in range(NSLOT):
                        W = (s + 1) * KPAN
                        KC = W // P
                        rinvs = []
                        for qs in range(2):
                            row = scores[s][:, qs, :]
                            mx = small.tile([P, 1], dt.float32, tag="mx")
                            nc.vector.reduce_max(
                                mx, row, axis=mybir.AxisListType.X)
                            bias_act = small.tile([P, 1], dt.float32, tag="ba")
                            nc.vector.tensor_scalar_mul(bias_act, mx, -SCALE)
                            lsum = small.tile([P, 1], dt.float32, tag="ls")
                            nc.scalar.activation(
                                out=row, in_=row,
                                func=mybir.ActivationFunctionType.Exp,
                                bias=bias_act, scale=SCALE, accum_out=lsum)
                            rinv = small.tile([P, 1], dt.float32, tag="ri")
                            nc.vector.reciprocal(rinv, lsum)
                            rinvs.append(rinv)
                        # transpose p -> pT (f32r) for the AV matmul
                        pt = ptpool.tile([P, 16, QBLK], dt.float32r, tag="pt")
                        for kc in range(KC):
                            for qs in range(2):
                                tps = psum_t.tile([P, P], dt.float32, tag="tp")
                                nc.tensor.transpose(
                                    tps,
                                    scores[s][:, qs, kc * P:(kc + 1) * P],
                                    ident)
                                copy_eng(kc + qs,
                                    pt[:, kc, qs * P:(qs + 1) * P], tps[:])
                        # AV: ctx[q, dout] accumulated over key chunks
                        ctx = [[psum_c.tile([P, 512], dt.float32, tag="ctx",
                                             name=f"ctx{s}_{qs}_{dh}")
                                for dh in range(2)] for qs in range(2)]
                        for kp in range(s + 1):
                            vp = vpool.tile([P, 4, D], dt.float32r, tag="vp")
                            nc.sync.dma_start(
                                vp[:], v_d[:, kp * 4:(kp + 1) * 4, :])
                            for qs in range(2):
                                for dh in range(2):
                                    for j in range(4):
                                        kc = kp * 4 + j
                                        nc.tensor.matmul(
                                            ctx[qs][dh],
                                            pt[:, kc, qs * P:(qs + 1) * P],
                                            vp[:, j, dh * 512:(dh + 1) * 512],
                                            start=(kc == 0), stop=(kc == KC - 1),
                                        )
                        for qs in range(2):
                            for dh in range(2):
                                oc = opool.tile([P, 512], dt.float32, tag="oc")
                                nc.vector.tensor_tensor(
                                    oc[:], ctx[qs][dh],
                                    rinvs[qs][:].to_broadcast((P, 512)),
                                    op=mybir.AluOpType.mult)
                                nc.sync.dma_start(
                                    out[s * QBLK + qs * P:
                                        s * QBLK + (qs + 1) * P,
                                        dh * 512:(dh + 1) * 512],
                                    oc[:])

            if reps > 1:
                with tc.For_i(0, reps):
                    body()
            else:
                body()

    nc.finalize()
    return nc


def make_core_inputs(x, Wq, Wk, Wv):
    """Slice/transform full inputs into 8 per-core input dicts."""
    in_maps = []
    qi = np.arange(QBLK)
    for c in range(8):
        b, par = c // 2, c % 2
        blocks = [2 * j + par for j in range(NSLOT)]
        xb = x[b]  # [S, D]
        xt = np.ascontiguousarray(xb.T)  # [D, S]
        qrows = np.concatenate(
            [np.arange(QBLK * blk, QBLK * (blk + 1)) for blk in blocks])
        xqt = np.ascontiguousarray(xb[qrows].T)  # [D, 1024]
        # additive bias for each slot's last key panel
        mb = np.zeros((NSLOT, 2, P, KPAN), np.float32)
        for s in range(NSLOT):
            bs = blocks[s]
            kidx = (COUNTS[s] - 1) * KPAN + np.arange(KPAN)[None, :]
            qidx = (QBLK * bs + qi)[:, None]
            bias = np.where(kidx <= qidx, 0.0, NEG).astype(np.float32)
            mb[s] = bias.reshape(2, P, KPAN)
        mb = np.ascontiguousarray(mb.transpose(2, 0, 1, 3))  # [P, slot, qs, k]
        in_maps.append({
            "xt": xt, "xqt": xqt, "wq": Wq, "wk": Wk, "wv": Wv, "mb": mb,
        })
    return in_maps


def assemble_output(results):
    out = np.empty((B, S, D), np.float32)
    for c in range(8):
        b, par = c // 2, c % 2
        blocks = [2 * j + par for j in range(NSLOT)]
        o = results[c]["out"]  # [1024, D]
        for s, blk in enumerate(blocks):
            out[b, QBLK * blk:QBLK * (blk + 1)] = o[QBLK * s:QBLK * (s + 1)]
    return out


def kernel(x, Wq, Wk, Wv):
    x = np.asarray(x, np.float32)
    Wq = np.asarray(Wq, np.float32)
    Wk = np.asarray(Wk, np.float32)
    Wv = np.asarray(Wv, np.float32)
    if "nc" not in _nc_cache:
        _nc_cache["nc"] = build_nc()
    nc = _nc_cache["nc"]
    in_maps = make_core_inputs(x, Wq, Wk, Wv)
    res = run_bass_kernel_spmd(nc, in_maps, core_ids=list(range(8)))
    return assemble_output(res.results)
